# revision 1
# baseline (speedup 1.0000x reference)
"""Block-sparse self-attention (DeepSpeed "fixed" layout) on 8 trn2 cores.

Problem: B=2, H=16, S=2048, D=64 fp32. Mask (identical for every head,
since numverts=1): each 64-wide diagonal window is dense, plus every 4th
16-col block ("stripe") is attended by all queries. Per 64-row query
window the attended key set = its 64 window cols + 512 stripe cols,
overlapping by 16 -> 560 distinct keys.

Sharding: 32 (b,h) pairs -> 4 per core (batch+head parallel).

Host prep per pair (pure layout + dtype cast):
  qT  [64, 2048]: Q^T.   kT [64, 2048]: K^T with columns reordered to
      [512 stripe cols | 32 windows x 48 non-stripe cols].
  vva [2048, 65]: V rows in the same reorder + a ones column (rides the
      PV matmul; lands the softmax denominator L in O' row 64).

On chip per pair (all matmul operands at base partition 0 — alternating
weight-load base partitions between instructions faults the device):
  S^T[k,q] = matmul(lhsT=K^T chunk, rhs=Q^T)          (PSUM fp32)
  P = exp(0.125 * S^T)  on ACT, fp16 -> SBUF           (scale fused)
  O'^T[65,q] += matmul(lhsT=V_aug chunk, rhs=P chunk)  (PSUM fp32)
  r = 1/L (row 64), broadcast across partitions, O = O'[0:64] * r
  out[pair] = O^T [64, 2048] fp32; host transposes back.
"""

import numpy as np

B, H, S, D = 2, 16, 2048, 64
NPAIRS = B * H
NCORES = 8
P_PER_CORE = NPAIRS // NCORES  # 4
NCH = 4        # stripe k-chunks of 128
NW = S // 64   # 32 windows
SCALE = float(D) ** -0.5


def _reorder_idx():
    blocks = np.arange(S // 16)
    stripe = blocks[blocks % 4 == 3]
    rest = blocks[blocks % 4 != 3]
    cols = np.arange(S).reshape(-1, 16)
    return np.concatenate([cols[stripe].ravel(), cols[rest].ravel()])


_REORDER = _reorder_idx()

_CACHE = {}


def _build(dt_in_name="float16", npairs=P_PER_CORE):
    from contextlib import ExitStack
    import concourse.bacc as bacc
    import concourse.tile as tile
    from concourse import mybir

    dt_in = getattr(mybir.dt, dt_in_name)
    f32 = mybir.dt.float32
    EXP = mybir.ActivationFunctionType.Exp

    nc = bacc.Bacc("TRN2", target_bir_lowering=False, debug=False,
                   num_devices=NCORES)
    qT = nc.dram_tensor("qT", [P_PER_CORE, 64, S], dt_in,
                        kind="ExternalInput").ap()
    kT = nc.dram_tensor("kT", [P_PER_CORE, 64, S], dt_in,
                        kind="ExternalInput").ap()
    vva = nc.dram_tensor("vva", [P_PER_CORE, S, 65], dt_in,
                         kind="ExternalInput").ap()
    out = nc.dram_tensor("out", [P_PER_CORE, 64, S], f32,
                         kind="ExternalOutput").ap()

    with tile.TileContext(nc) as tc, ExitStack() as ctx:
        qk_pool = ctx.enter_context(tc.tile_pool(name="qk", bufs=2))
        v_pool = ctx.enter_context(tc.tile_pool(name="v", bufs=2))
        p_pool = ctx.enter_context(tc.tile_pool(name="p", bufs=2))
        n_pool = ctx.enter_context(tc.tile_pool(name="n", bufs=2))
        s_pool = ctx.enter_context(tc.tile_pool(name="s", bufs=2, space="PSUM"))
        o_pool = ctx.enter_context(tc.tile_pool(name="o", bufs=1, space="PSUM"))

        for p in range(npairs):
            qt = qk_pool.tile([64, S], dt_in, tag="q")
            nc.sync.dma_start(out=qt, in_=qT[p])
            kt = qk_pool.tile([64, S], dt_in, tag="k")
            nc.sync.dma_start(out=kt, in_=kT[p])
            vs = v_pool.tile([128, NCH, 65], dt_in, tag="vs")
            nc.sync.dma_start(
                out=vs, in_=vva[p, 0:512].rearrange("(c r) d -> r c d", r=128))
            vw = v_pool.tile([48, NW * 65], dt_in, tag="vw")
            vw3 = vw.rearrange("j (w d) -> j w d", d=65)
            nc.sync.dma_start(
                out=vw3, in_=vva[p, 512:S].rearrange("(w j) d -> j w d", j=48))

            ps = p_pool.tile([128, NCH, S], dt_in, tag="ps")
            pw = p_pool.tile([48, NW * 64], dt_in, tag="pw")

            # stripe scores + exp, in [128, 1024] PSUM tiles (2 banks each)
            for c in range(NCH):
                for h in range(2):
                    st = s_pool.tile([128, 1024], f32, tag="s")
                    for g in range(2):
                        q0 = h * 1024 + g * 512
                        nc.tensor.matmul(
                            out=st[:, g * 512:(g + 1) * 512],
                            lhsT=kt[:, c * 128:(c + 1) * 128],
                            rhs=qt[:, q0:q0 + 512],
                            start=True, stop=True)
                    nc.scalar.activation(
                        out=ps[:, c, h * 1024:(h + 1) * 1024], in_=st,
                        func=EXP, scale=SCALE)

            # window scores: window w -> partitions 0:48, free offset 64*(w%16)
            for h in range(2):
                sw = s_pool.tile([48, 1024], f32, tag="s")
                for w in range(h * 16, h * 16 + 16):
                    fo = (w - h * 16) * 64
                    nc.tensor.matmul(
                        out=sw[:, fo:fo + 64],
                        lhsT=kt[:, 512 + 48 * w:512 + 48 * w + 48],
                        rhs=qt[:, 64 * w:64 * w + 64],
                        start=True, stop=True)
                nc.scalar.activation(
                    out=pw[:, h * 1024:(h + 1) * 1024], in_=sw,
                    func=EXP, scale=SCALE)

            # PV: accumulate O'^T [65, q] over 4 stripe chunks + windows
            ov = o_pool.tile([65, S], f32, tag="o")
            for g in range(4):
                q0 = g * 512
                for c in range(NCH):
                    nc.tensor.matmul(
                        out=ov[:, q0:q0 + 512],
                        lhsT=vs[:, c, :],
                        rhs=ps[:, c, q0:q0 + 512],
                        start=(c == 0), stop=False, skip_group_check=True)
            for w in range(NW):
                nc.tensor.matmul(
                    out=ov[:, 64 * w:64 * w + 64],
                    lhsT=vw[:, 65 * w:65 * w + 65],
                    rhs=pw[:, 64 * w:64 * w + 64],
                    start=False, stop=(w == NW - 1), skip_group_check=True)

            # normalize: r = 1/L, broadcast, multiply. The L row sits at
            # PSUM partition 64; custom-DVE ops misread nonzero base
            # partitions on HW, so: native copy to SBUF@64, DMA to
            # partition 0, reciprocal there, then broadcast.
            lt = n_pool.tile([65, S], f32, tag="l")
            nc.vector.tensor_copy(lt[64:65], ov[64:65, :])
            rt = n_pool.tile([1, S], f32, tag="r")
            nc.sync.dma_start(out=rt, in_=lt[64:65])
            rr = n_pool.tile([1, S], f32, tag="rr")
            nc.vector.reciprocal_approx_fast(out=rr, in_=rt)
            rb = n_pool.tile([64, S], f32, tag="rb")
            nc.gpsimd.partition_broadcast(rb, rr[0:1])
            ob = n_pool.tile([64, S], f32, tag="ob")
            nc.vector.tensor_mul(out=ob, in0=ov[0:64, :], in1=rb)
            nc.sync.dma_start(out=out[p], in_=ob)

    nc.compile()
    return nc


def _get_nc(dt_in_name="float16"):
    if dt_in_name not in _CACHE:
        _CACHE[dt_in_name] = _build(dt_in_name)
    return _CACHE[dt_in_name]


def _prep_inputs(query, key, value, np_dt):
    q = np.asarray(query).reshape(NPAIRS, S, D)
    k = np.asarray(key).reshape(NPAIRS, S, D)
    v = np.asarray(value).reshape(NPAIRS, S, D)
    kr = k[:, _REORDER, :]
    vr = v[:, _REORDER, :]
    qT = np.ascontiguousarray(q.transpose(0, 2, 1)).astype(np_dt)
    kT = np.ascontiguousarray(kr.transpose(0, 2, 1)).astype(np_dt)
    vva = np.concatenate(
        [vr, np.ones((NPAIRS, S, 1), vr.dtype)], axis=2).astype(np_dt)
    in_maps = []
    for core in range(NCORES):
        sl = slice(core * P_PER_CORE, (core + 1) * P_PER_CORE)
        in_maps.append({"qT": np.ascontiguousarray(qT[sl]),
                        "kT": np.ascontiguousarray(kT[sl]),
                        "vva": np.ascontiguousarray(vva[sl])})
    return in_maps


def _run(query, key, value, dt_in_name="float16", trace=False):
    from concourse.bass_utils import run_bass_kernel_spmd
    nc = _get_nc(dt_in_name)
    in_maps = _prep_inputs(query, key, value, np.float16
                           if dt_in_name == "float16" else np.float32)
    res = run_bass_kernel_spmd(nc, in_maps, list(range(NCORES)), trace=trace)
    o = np.concatenate([res.results[i]["out"] for i in range(NCORES)], axis=0)
    full = o.transpose(0, 2, 1).reshape(B, H, S, D).astype(np.float32)
    return full, res


def kernel(query, key, value):
    full, _ = _run(np.asarray(query), np.asarray(key), np.asarray(value))
    return full



# revision 12
# speedup vs baseline: 1.2860x; 1.2860x over previous
"""Block-sparse self-attention (DeepSpeed "fixed" layout) on 8 trn2 cores.

Problem: B=2, H=16, S=2048, D=64 fp32. Mask (identical for every head,
numverts=1): each 64-wide diagonal window is dense, plus every 4th
16-col block ("stripe") is attended by all queries. Per 64-row query
window the attended key set = 512 stripe cols + 48 non-stripe window
cols.

Sharding: 32 (b,h) pairs -> 4 per core (batch+head parallel).

Host prep per pair (pure layout + dtype cast; reorder puts the 512
stripe cols first, then 32 windows x 48 non-stripe cols):
  qT  [64, 2048]  Q^T
  kT  [64, 2560]  K^T reordered: 512 stripe cols, then 16 window-pairs
                  of 128 cols each [48 even | 16 zero | 48 odd | 16 zero]
                  (zero padding keeps engine partition bases 32-aligned)
  vs  [128, 4*65] stripe V_aug in on-chip layout: partition r, chunk c
                  holds V[reorder[c*128+r]] ++ [1]  (ones col -> softmax
                  denominator L rides the PV matmul)
  vw2 [112, 16*65] window-pair V_aug: partitions 0:48 = window 2i,
                  64:112 = window 2i+1 (48:64 zero), ones col each

On chip per pair (everything in plain 128x128 PE mode, fp16 operands):
  stripe scores  S^T[k,q] = matmul(kt chunk, qt)      16x [128,512]
  window scores  [96,128] blocks: windows (2j, 2j+1) stacked on the
                 output-partition axis share one matmul; the off-
                 diagonal cross-window entries are computed but thrown
                 away by the block-diagonal exp-write into a zeroed P
  P = exp(0.125 * S) via ACT, fp32 PSUM -> fp16 SBUF
  O'^T[65,q] accumulates stripe chunks ([65,512] x16) and window pairs
                 ([65,128] x16); row 64 = L (ones columns of V_aug)
  out[p] = O' [65, 2048] fp32, DMA'd straight from PSUM
Host: O = (O'[0:64] / O'[64])^T per pair.
"""

import numpy as np

B, H, S, D = 2, 16, 2048, 64
NPAIRS = B * H
NCORES = 8
P_PER_CORE = NPAIRS // NCORES  # 4
NCH = 4        # stripe k-chunks of 128
NW = S // 64   # 32 windows
SCALE = float(D) ** -0.5


def _reorder_idx():
    blocks = np.arange(S // 16)
    stripe = blocks[blocks % 4 == 3]
    rest = blocks[blocks % 4 != 3]
    cols = np.arange(S).reshape(-1, 16)
    return np.concatenate([cols[stripe].ravel(), cols[rest].ravel()])


_REORDER = _reorder_idx()

_CACHE = {}


def _build(dt_in_name="float16", npairs=P_PER_CORE):
    from contextlib import ExitStack
    import concourse.bacc as bacc
    import concourse.tile as tile
    from concourse import mybir

    dt_in = getattr(mybir.dt, dt_in_name)
    f32 = mybir.dt.float32
    i16 = mybir.dt.int16
    EXP = mybir.ActivationFunctionType.Exp
    MUL = mybir.AluOpType.mult
    ADD = mybir.AluOpType.add
    # Schraudolph exp in fp16 bit space: fp16_bits(exp(s*SCALE)) ~
    # s * (SCALE*1024*log2 e) + (15*1024 - 59.3). One DVE tensor_scalar
    # (fp32 PSUM -> int16 convert) per tile; the int16 buffer is the fp16
    # P tile by bitcast. ~1.5% rms elementwise, applied to ~30% of P.
    SCH_A = SCALE * 1024.0 / float(np.log(2.0))
    SCH_B = 15.0 * 1024.0 - 59.3

    nc = bacc.Bacc("TRN2", target_bir_lowering=False, debug=False,
                   num_devices=NCORES)
    qT = nc.dram_tensor("qT", [npairs, 64, S], dt_in,
                        kind="ExternalInput").ap()
    kT = nc.dram_tensor("kT", [npairs, 64, 512 + 64 * NW], dt_in,
                        kind="ExternalInput").ap()
    vs = nc.dram_tensor("vs", [npairs, 128, NCH * 65], dt_in,
                        kind="ExternalInput").ap()
    vw2 = nc.dram_tensor("vw2", [npairs, 112, (NW // 2) * 65], dt_in,
                         kind="ExternalInput").ap()
    out = nc.dram_tensor("out", [npairs, 65, S], f32,
                         kind="ExternalOutput").ap()

    with tile.TileContext(nc) as tc, ExitStack() as ctx:
        qk_pool = ctx.enter_context(tc.tile_pool(name="qk", bufs=2))
        v_pool = ctx.enter_context(tc.tile_pool(name="v", bufs=2))
        p_pool = ctx.enter_context(tc.tile_pool(name="p", bufs=2))
        s_pool = ctx.enter_context(tc.tile_pool(name="s", bufs=2, space="PSUM"))
        o_pool = ctx.enter_context(tc.tile_pool(name="o", bufs=1, space="PSUM"))

        for p in range(npairs):
            qt = qk_pool.tile([64, S], dt_in, tag="q")
            nc.sync.dma_start(out=qt, in_=qT[p])
            kt = qk_pool.tile([64, 512 + 64 * NW], dt_in, tag="k")
            nc.sync.dma_start(out=kt, in_=kT[p])
            vst = v_pool.tile([128, NCH * 65], dt_in, tag="vs")
            nc.sync.dma_start(out=vst, in_=vs[p])
            vwt = v_pool.tile([112, (NW // 2) * 65], dt_in, tag="vw")
            nc.sync.dma_start(out=vwt, in_=vw2[p])

            ps = p_pool.tile([128, NCH, S], dt_in, tag="ps")
            pw = p_pool.tile([112, S], dt_in, tag="pw")
            # zero P-window so the cross-window blocks of the stacked
            # window matmuls contribute nothing
            nc.gpsimd.memset(pw, 0.0)

            # stripe scores + exp: chunk c, q-half g -> [128, 1024].
            # Chunk 3 exps on DVE (Schraudolph) to unload the ACT engine.
            for c in range(NCH):
                for g in range(2):
                    st = s_pool.tile([128, 1024], f32, tag="s")
                    for u in range(2):
                        q0 = g * 1024 + u * 512
                        nc.tensor.matmul(
                            out=st[:, u * 512:(u + 1) * 512],
                            lhsT=kt[:, c * 128:(c + 1) * 128],
                            rhs=qt[:, q0:q0 + 512],
                            start=True, stop=True)
                    po = ps[:, c, g * 1024:(g + 1) * 1024]
                    if c == 3:
                        nc.vector.tensor_scalar(
                            out=po.bitcast(i16), in0=st,
                            scalar1=SCH_A, scalar2=SCH_B, op0=MUL, op1=ADD)
                    else:
                        nc.scalar.activation(out=po, in_=st,
                                             func=EXP, scale=SCALE)

            # window scores: window-pair j=(2j, 2j+1) -> [112, 128] block
            # (includes invalid cross-window entries), 8 pairs per buffer
            for h in range(2):
                sw = s_pool.tile([128, 1024], f32, tag="s")
                for j in range(8 * h, 8 * h + 8):
                    fo = (j - 8 * h) * 128
                    nc.tensor.matmul(
                        out=sw[0:112, fo:fo + 128],
                        lhsT=kt[:, 512 + 128 * j:512 + 128 * j + 112],
                        rhs=qt[:, 128 * j:128 * j + 128],
                        start=True, stop=True)
                # exp only the diagonal blocks into the zeroed pw (DVE)
                sw4 = sw.rearrange("p (j t f) -> p j t f", t=2, f=64)
                pw4 = pw.rearrange("p (j t f) -> p j t f", t=2, f=64)
                nc.vector.tensor_scalar(
                    out=pw4[0:48, 8 * h:8 * h + 8, 0, :].bitcast(i16),
                    in0=sw4[0:48, 0:8, 0, :],
                    scalar1=SCH_A, scalar2=SCH_B, op0=MUL, op1=ADD)
                nc.vector.tensor_scalar(
                    out=pw4[64:112, 8 * h:8 * h + 8, 1, :].bitcast(i16),
                    in0=sw4[64:112, 0:8, 1, :],
                    scalar1=SCH_A, scalar2=SCH_B, op0=MUL, op1=ADD)

            # PV: accumulate O'^T [65, q]: 4 stripe chunks + 16 window
            # pairs; ones col of V_aug lands L in row 64
            ov = o_pool.tile([128, S], f32, tag="o")
            for g in range(4):
                q0 = g * 512
                for c in range(NCH):
                    nc.tensor.matmul(
                        out=ov[0:65, q0:q0 + 512],
                        lhsT=vst[:, c * 65:(c + 1) * 65],
                        rhs=ps[:, c, q0:q0 + 512],
                        start=(c == 0), stop=False, skip_group_check=True)
            for j in range(NW // 2):
                nc.tensor.matmul(
                    out=ov[0:65, 128 * j:128 * j + 128],
                    lhsT=vwt[:, j * 65:(j + 1) * 65],
                    rhs=pw[0:112, 128 * j:128 * j + 128],
                    start=False, stop=(j == NW // 2 - 1),
                    skip_group_check=True)

            ob = p_pool.tile([65, S], f32, tag="ob")
            nc.vector.tensor_copy(ob, ov[0:65, :])
            nc.sync.dma_start(out=out[p], in_=ob)

    nc.compile()
    return nc


def _get_nc(dt_in_name="float16"):
    if dt_in_name not in _CACHE:
        _CACHE[dt_in_name] = _build(dt_in_name)
    return _CACHE[dt_in_name]


def _prep_inputs(query, key, value, np_dt):
    q = np.asarray(query).reshape(NPAIRS, S, D)
    k = np.asarray(key).reshape(NPAIRS, S, D)
    v = np.asarray(value).reshape(NPAIRS, S, D)
    kr = k[:, _REORDER, :]
    vr = v[:, _REORDER, :]
    qT = np.ascontiguousarray(q.transpose(0, 2, 1)).astype(np_dt)
    # kT: 512 stripe cols, then window-pairs padded to 128 cols each
    # [48 even | 16 zero | 48 odd | 16 zero]
    kTs = kr.transpose(0, 2, 1).astype(np_dt)  # [P, 64, 2048]
    kT = np.zeros((NPAIRS, 64, 512 + 64 * NW), np_dt)
    kT[:, :, 0:512] = kTs[:, :, 0:512]
    kw = kTs[:, :, 512:].reshape(NPAIRS, 64, NW // 2, 2, 48)
    kTw = kT[:, :, 512:].reshape(NPAIRS, 64, NW // 2, 2, 64)
    kTw[:, :, :, :, 0:48] = kw
    va = np.concatenate(
        [vr, np.ones((NPAIRS, S, 1), vr.dtype)], axis=2).astype(np_dt)
    # stripe V_aug: [pair, partition r, chunk c, 65]
    vs = np.ascontiguousarray(
        va[:, :512].reshape(NPAIRS, NCH, 128, 65).transpose(0, 2, 1, 3)
    ).reshape(NPAIRS, 128, NCH * 65)
    # window-pair V_aug: [pair, 112, pairidx i, 65]: rows 0:48 window 2i,
    # rows 64:112 window 2i+1, rows 48:64 zero
    vw = va[:, 512:].reshape(NPAIRS, NW // 2, 2, 48, 65)
    vw2 = np.zeros((NPAIRS, 112, NW // 2, 65), np_dt)
    vw2[:, 0:48] = vw[:, :, 0].transpose(0, 2, 1, 3)
    vw2[:, 64:112] = vw[:, :, 1].transpose(0, 2, 1, 3)
    vw2 = np.ascontiguousarray(vw2).reshape(NPAIRS, 112, (NW // 2) * 65)
    in_maps = []
    for core in range(NCORES):
        sl = slice(core * P_PER_CORE, (core + 1) * P_PER_CORE)
        in_maps.append({"qT": np.ascontiguousarray(qT[sl]),
                        "kT": np.ascontiguousarray(kT[sl]),
                        "vs": np.ascontiguousarray(vs[sl]),
                        "vw2": np.ascontiguousarray(vw2[sl])})
    return in_maps


def _run(query, key, value, dt_in_name="float16", trace=False):
    from concourse.bass_utils import run_bass_kernel_spmd
    nc = _get_nc(dt_in_name)
    in_maps = _prep_inputs(query, key, value, np.float16
                           if dt_in_name == "float16" else np.float32)
    res = run_bass_kernel_spmd(nc, in_maps, list(range(NCORES)), trace=trace)
    o = np.concatenate([res.results[i]["out"] for i in range(NCORES)], axis=0)
    full = (o[:, 0:64, :] / o[:, 64:65, :]).transpose(0, 2, 1).reshape(
        B, H, S, D).astype(np.float32)
    return full, res


def kernel(query, key, value):
    full, _ = _run(np.asarray(query), np.asarray(key), np.asarray(value))
    return full


# revision 15
# speedup vs baseline: 1.2883x; 1.0017x over previous
"""Block-sparse self-attention (DeepSpeed "fixed" layout) on 8 trn2 cores.

Problem: B=2, H=16, S=2048, D=64 fp32. Mask (identical for every head,
numverts=1): each 64-wide diagonal window is dense, plus every 4th
16-col block ("stripe") is attended by all queries. Per 64-row query
window the attended key set = 512 stripe cols + 48 non-stripe window
cols.

Sharding: 32 (b,h) pairs -> 4 per core (batch+head parallel).

Host prep per pair (pure layout + dtype cast; reorder puts the 512
stripe cols first, then 32 windows x 48 non-stripe cols):
  qT  [64, 2048]  Q^T
  kT  [64, 2560]  K^T reordered: 512 stripe cols, then 16 window-pairs
                  of 128 cols each [48 even | 16 zero | 48 odd | 16 zero]
                  (zero padding keeps engine partition bases 32-aligned)
  vs  [128, 4*65] stripe V_aug in on-chip layout: partition r, chunk c
                  holds V[reorder[c*128+r]] ++ [1]  (ones col -> softmax
                  denominator L rides the PV matmul)
  vw2 [112, 16*65] window-pair V_aug: partitions 0:48 = window 2i,
                  64:112 = window 2i+1 (48:64 zero), ones col each

On chip per pair (everything in plain 128x128 PE mode, fp16 operands):
  stripe scores  S^T[k,q] = matmul(kt chunk, qt)      16x [128,512]
  window scores  [96,128] blocks: windows (2j, 2j+1) stacked on the
                 output-partition axis share one matmul; the off-
                 diagonal cross-window entries are computed but thrown
                 away by the block-diagonal exp-write into a zeroed P
  P = exp(0.125 * S) via ACT, fp32 PSUM -> fp16 SBUF
  O'^T[65,q] accumulates stripe chunks ([65,512] x16) and window pairs
                 ([65,128] x16); row 64 = L (ones columns of V_aug)
  out[p] = O' [65, 2048] fp32, DMA'd straight from PSUM
Host: O = (O'[0:64] / O'[64])^T per pair.
"""

import numpy as np

B, H, S, D = 2, 16, 2048, 64
NPAIRS = B * H
NCORES = 8
P_PER_CORE = NPAIRS // NCORES  # 4
NCH = 4        # stripe k-chunks of 128
NW = S // 64   # 32 windows
SCALE = float(D) ** -0.5


def _reorder_idx():
    blocks = np.arange(S // 16)
    stripe = blocks[blocks % 4 == 3]
    rest = blocks[blocks % 4 != 3]
    cols = np.arange(S).reshape(-1, 16)
    return np.concatenate([cols[stripe].ravel(), cols[rest].ravel()])


_REORDER = _reorder_idx()

_CACHE = {}


def _build(dt_in_name="float16", npairs=P_PER_CORE):
    from contextlib import ExitStack
    import concourse.bacc as bacc
    import concourse.tile as tile
    from concourse import mybir

    dt_in = getattr(mybir.dt, dt_in_name)
    f32 = mybir.dt.float32
    i16 = mybir.dt.int16
    EXP = mybir.ActivationFunctionType.Exp
    MUL = mybir.AluOpType.mult
    ADD = mybir.AluOpType.add
    # Schraudolph exp in fp16 bit space: fp16_bits(exp(s*SCALE)) ~
    # s * (SCALE*1024*log2 e) + (15*1024 - 59.3). One DVE tensor_scalar
    # (fp32 PSUM -> int16 convert) per tile; the int16 buffer is the fp16
    # P tile by bitcast. ~1.5% rms elementwise, applied to ~30% of P.
    SCH_A = SCALE * 1024.0 / float(np.log(2.0))
    SCH_B = 15.0 * 1024.0 - 59.3

    nc = bacc.Bacc("TRN2", target_bir_lowering=False, debug=False,
                   num_devices=NCORES)
    qT = nc.dram_tensor("qT", [npairs, 64, S], dt_in,
                        kind="ExternalInput").ap()
    kT = nc.dram_tensor("kT", [npairs, 64, 512 + 64 * NW], dt_in,
                        kind="ExternalInput").ap()
    vs = nc.dram_tensor("vs", [npairs, 128, NCH * 65], dt_in,
                        kind="ExternalInput").ap()
    vw2 = nc.dram_tensor("vw2", [npairs, 112, (NW // 2) * 65], dt_in,
                         kind="ExternalInput").ap()
    out = nc.dram_tensor("out", [npairs, 65, S], f32,
                         kind="ExternalOutput").ap()

    COPY = mybir.ActivationFunctionType.Copy

    with tile.TileContext(nc) as tc, ExitStack() as ctx:
        qk_pool = ctx.enter_context(tc.tile_pool(name="qk", bufs=2))
        v_pool = ctx.enter_context(tc.tile_pool(name="v", bufs=2))
        p_pool = ctx.enter_context(tc.tile_pool(name="p", bufs=2))
        s_pool = ctx.enter_context(tc.tile_pool(name="s", bufs=2, space="PSUM"))
        o_pool = ctx.enter_context(tc.tile_pool(name="o", bufs=2, space="PSUM"))

        def load_tiles(p):
            qt = qk_pool.tile([64, S], dt_in, tag="q")
            nc.sync.dma_start(out=qt, in_=qT[p])
            kt = qk_pool.tile([64, 512 + 64 * NW], dt_in, tag="k")
            nc.sync.dma_start(out=kt, in_=kT[p])
            vst = v_pool.tile([128, NCH * 65], dt_in, tag="vs")
            nc.sync.dma_start(out=vst, in_=vs[p])
            vwt = v_pool.tile([112, (NW // 2) * 65], dt_in, tag="vw")
            nc.sync.dma_start(out=vwt, in_=vw2[p])
            ps = p_pool.tile([128, NCH, S], dt_in, tag="ps")
            pw = p_pool.tile([112, S], dt_in, tag="pw")
            # zero P-window so the cross-window blocks of the stacked
            # window matmuls contribute nothing
            nc.gpsimd.memset(pw, 0.0)
            return dict(p=p, qt=qt, kt=kt, vst=vst, vwt=vwt, ps=ps, pw=pw)

        def pv_step(cx, i):
            # i in 0..31: per q-half h: 8 stripe MMs then 8 window MMs.
            # O'^T accumulates in a [65, 1024] half; V_aug ones col lands
            # the softmax denominator L in row 64.
            h, r = i // 16, i % 16
            if r == 0:
                cx["ov" + str(h)] = o_pool.tile([128, 1024], f32, tag="o",
                                                name=f"ov{cx['p']}_{h}")
            ov = cx["ov" + str(h)]
            if r < 8:
                gl, c = 2 * h + r // 4, r % 4
                q0 = (gl % 2) * 512
                nc.tensor.matmul(
                    out=ov[0:65, q0:q0 + 512],
                    lhsT=cx["vst"][:, c * 65:(c + 1) * 65],
                    rhs=cx["ps"][:, c, gl * 512:(gl + 1) * 512],
                    start=(c == 0), stop=False, skip_group_check=True)
            else:
                j = 8 * h + (r - 8)
                q0 = (j % 8) * 128
                nc.tensor.matmul(
                    out=ov[0:65, q0:q0 + 128],
                    lhsT=cx["vwt"][:, j * 65:(j + 1) * 65],
                    rhs=cx["pw"][0:112, 128 * j:128 * j + 128],
                    start=False, stop=(r == 15), skip_group_check=True)

        def pv_copy(cx, h):
            # PSUM -> SBUF staging (DMA cannot read PSUM); h0 via ACT,
            # h1 via DVE to balance the two drain engines
            ov = cx["ov" + str(h)]
            ob = p_pool.tile([65, 1024], f32, tag="ob")
            if h == 0:
                nc.scalar.activation(out=ob, in_=ov[0:65, :], func=COPY)
            else:
                nc.vector.tensor_copy(ob, ov[0:65, :])
            nc.sync.dma_start(
                out=out[cx["p"], :, h * 1024:(h + 1) * 1024], in_=ob)

        ctxs = [load_tiles(0)]
        for p in range(npairs):
            nxt_needed = p + 1 < npairs
            cur = ctxs[p]
            prev = ctxs[p - 1] if p > 0 else None
            # emit, prefetching next pair's tiles after the first round
            qt, kt, ps, pw = cur["qt"], cur["kt"], cur["ps"], cur["pw"]
            for r in range(8):
                c, g = r // 2, r % 2
                st = s_pool.tile([128, 1024], f32, tag="s")
                for u in range(2):
                    q0 = g * 1024 + u * 512
                    nc.tensor.matmul(
                        out=st[:, u * 512:(u + 1) * 512],
                        lhsT=kt[:, c * 128:(c + 1) * 128],
                        rhs=qt[:, q0:q0 + 512],
                        start=True, stop=True)
                po = ps[:, c, g * 1024:(g + 1) * 1024]
                if c == 3:
                    nc.vector.tensor_scalar(
                        out=po.bitcast(i16), in0=st,
                        scalar1=SCH_A, scalar2=SCH_B, op0=MUL, op1=ADD)
                else:
                    nc.scalar.activation(out=po, in_=st,
                                         func=EXP, scale=SCALE)
                if r == 0 and nxt_needed:
                    ctxs.append(load_tiles(p + 1))
                if prev is not None:
                    for i in range(4 * r, 4 * r + 4):
                        pv_step(prev, i)
                    if r == 3:
                        pv_copy(prev, 0)
                    if r == 7:
                        pv_copy(prev, 1)
            for h in range(2):
                sw = s_pool.tile([128, 1024], f32, tag="s")
                for j in range(8 * h, 8 * h + 8):
                    fo = (j - 8 * h) * 128
                    nc.tensor.matmul(
                        out=sw[0:112, fo:fo + 128],
                        lhsT=kt[:, 512 + 128 * j:512 + 128 * j + 112],
                        rhs=qt[:, 128 * j:128 * j + 128],
                        start=True, stop=True)
                sw4 = sw.rearrange("p (j t f) -> p j t f", t=2, f=64)
                pw4 = pw.rearrange("p (j t f) -> p j t f", t=2, f=64)
                nc.vector.tensor_scalar(
                    out=pw4[0:48, 8 * h:8 * h + 8, 0, :].bitcast(i16),
                    in0=sw4[0:48, 0:8, 0, :],
                    scalar1=SCH_A, scalar2=SCH_B, op0=MUL, op1=ADD)
                nc.vector.tensor_scalar(
                    out=pw4[64:112, 8 * h:8 * h + 8, 1, :].bitcast(i16),
                    in0=sw4[64:112, 0:8, 1, :],
                    scalar1=SCH_A, scalar2=SCH_B, op0=MUL, op1=ADD)
        # flush: PV of the last pair
        last = ctxs[-1]
        for i in range(32):
            pv_step(last, i)
            if i == 15:
                pv_copy(last, 0)
        pv_copy(last, 1)

    nc.compile()
    return nc


def _get_nc(dt_in_name="float16"):
    if dt_in_name not in _CACHE:
        _CACHE[dt_in_name] = _build(dt_in_name)
    return _CACHE[dt_in_name]


def _prep_inputs(query, key, value, np_dt):
    q = np.asarray(query).reshape(NPAIRS, S, D)
    k = np.asarray(key).reshape(NPAIRS, S, D)
    v = np.asarray(value).reshape(NPAIRS, S, D)
    kr = k[:, _REORDER, :]
    vr = v[:, _REORDER, :]
    qT = np.ascontiguousarray(q.transpose(0, 2, 1)).astype(np_dt)
    # kT: 512 stripe cols, then window-pairs padded to 128 cols each
    # [48 even | 16 zero | 48 odd | 16 zero]
    kTs = kr.transpose(0, 2, 1).astype(np_dt)  # [P, 64, 2048]
    kT = np.zeros((NPAIRS, 64, 512 + 64 * NW), np_dt)
    kT[:, :, 0:512] = kTs[:, :, 0:512]
    kw = kTs[:, :, 512:].reshape(NPAIRS, 64, NW // 2, 2, 48)
    kTw = kT[:, :, 512:].reshape(NPAIRS, 64, NW // 2, 2, 64)
    kTw[:, :, :, :, 0:48] = kw
    va = np.concatenate(
        [vr, np.ones((NPAIRS, S, 1), vr.dtype)], axis=2).astype(np_dt)
    # stripe V_aug: [pair, partition r, chunk c, 65]
    vs = np.ascontiguousarray(
        va[:, :512].reshape(NPAIRS, NCH, 128, 65).transpose(0, 2, 1, 3)
    ).reshape(NPAIRS, 128, NCH * 65)
    # window-pair V_aug: [pair, 112, pairidx i, 65]: rows 0:48 window 2i,
    # rows 64:112 window 2i+1, rows 48:64 zero
    vw = va[:, 512:].reshape(NPAIRS, NW // 2, 2, 48, 65)
    vw2 = np.zeros((NPAIRS, 112, NW // 2, 65), np_dt)
    vw2[:, 0:48] = vw[:, :, 0].transpose(0, 2, 1, 3)
    vw2[:, 64:112] = vw[:, :, 1].transpose(0, 2, 1, 3)
    vw2 = np.ascontiguousarray(vw2).reshape(NPAIRS, 112, (NW // 2) * 65)
    in_maps = []
    for core in range(NCORES):
        sl = slice(core * P_PER_CORE, (core + 1) * P_PER_CORE)
        in_maps.append({"qT": np.ascontiguousarray(qT[sl]),
                        "kT": np.ascontiguousarray(kT[sl]),
                        "vs": np.ascontiguousarray(vs[sl]),
                        "vw2": np.ascontiguousarray(vw2[sl])})
    return in_maps


def _run(query, key, value, dt_in_name="float16", trace=False):
    from concourse.bass_utils import run_bass_kernel_spmd
    nc = _get_nc(dt_in_name)
    in_maps = _prep_inputs(query, key, value, np.float16
                           if dt_in_name == "float16" else np.float32)
    res = run_bass_kernel_spmd(nc, in_maps, list(range(NCORES)), trace=trace)
    o = np.concatenate([res.results[i]["out"] for i in range(NCORES)], axis=0)
    full = (o[:, 0:64, :] / o[:, 64:65, :]).transpose(0, 2, 1).reshape(
        B, H, S, D).astype(np.float32)
    return full, res


def kernel(query, key, value):
    full, _ = _run(np.asarray(query), np.asarray(key), np.asarray(value))
    return full


# revision 19
# speedup vs baseline: 1.4224x; 1.1041x over previous
"""Block-sparse self-attention (DeepSpeed "fixed" layout) on 8 trn2 cores.

Problem: B=2, H=16, S=2048, D=64 fp32. Mask (identical for every head,
numverts=1): each 64-wide diagonal window is dense, plus every 4th
16-col block ("stripe") is attended by all queries. Per 64-row query
window the attended key set = 512 stripe cols + 48 non-stripe window
cols.

Sharding: 32 (b,h) pairs -> 4 per core (batch+head parallel).

Host prep per pair (pure layout + dtype cast; reorder puts the 512
stripe cols first, then 32 windows x 48 non-stripe cols):
  qT  [64, 2048]  Q^T
  kT  [64, 2560]  K^T reordered: 512 stripe cols, then 16 window-pairs
                  of 128 cols each [48 even | 16 zero | 48 odd | 16 zero]
                  (zero padding keeps engine partition bases 32-aligned)
  vs  [128, 4*65] stripe V_aug in on-chip layout: partition r, chunk c
                  holds V[reorder[c*128+r]] ++ [1]  (ones col -> softmax
                  denominator L rides the PV matmul)
  vw2 [112, 16*65] window-pair V_aug: partitions 0:48 = window 2i,
                  64:112 = window 2i+1 (48:64 zero), ones col each

On chip per pair (everything in plain 128x128 PE mode, fp16 operands):
  stripe scores  S^T[k,q] = matmul(kt chunk, qt)      16x [128,512]
  window scores  [96,128] blocks: windows (2j, 2j+1) stacked on the
                 output-partition axis share one matmul; the off-
                 diagonal cross-window entries are computed but thrown
                 away by the block-diagonal exp-write into a zeroed P
  P = exp(0.125 * S) via ACT, fp32 PSUM -> fp16 SBUF
  O'^T[65,q] accumulates stripe chunks ([65,512] x16) and window pairs
                 ([65,128] x16); row 64 = L (ones columns of V_aug)
  out[p] = O' [65, 2048] fp32, DMA'd straight from PSUM
Host: O = (O'[0:64] / O'[64])^T per pair.
"""

import numpy as np

B, H, S, D = 2, 16, 2048, 64
NPAIRS = B * H
NCORES = 8
P_PER_CORE = NPAIRS // NCORES  # 4
NCH = 4        # stripe k-chunks of 128
NW = S // 64   # 32 windows
SCALE = float(D) ** -0.5


def _reorder_idx():
    blocks = np.arange(S // 16)
    stripe = blocks[blocks % 4 == 3]
    rest = blocks[blocks % 4 != 3]
    cols = np.arange(S).reshape(-1, 16)
    return np.concatenate([cols[stripe].ravel(), cols[rest].ravel()])


_REORDER = _reorder_idx()

_CACHE = {}


def _build(dt_in_name="float16", npairs=P_PER_CORE):
    from contextlib import ExitStack
    import concourse.bacc as bacc
    import concourse.tile as tile
    from concourse import mybir

    dt_in = getattr(mybir.dt, dt_in_name)
    f32 = mybir.dt.float32
    i16 = mybir.dt.int16
    EXP = mybir.ActivationFunctionType.Exp
    MUL = mybir.AluOpType.mult
    ADD = mybir.AluOpType.add
    # Schraudolph exp in fp16 bit space: fp16_bits(exp(s*SCALE)) ~
    # s * (SCALE*1024*log2 e) + (15*1024 - 59.3). One DVE tensor_scalar
    # (fp32 PSUM -> int16 convert) per tile; the int16 buffer is the fp16
    # P tile by bitcast. ~1.5% rms elementwise, applied to ~30% of P.
    SCH_A = SCALE * 1024.0 / float(np.log(2.0))
    SCH_B = 15.0 * 1024.0 - 59.3

    nc = bacc.Bacc("TRN2", target_bir_lowering=False, debug=False,
                   num_devices=NCORES)
    qT = nc.dram_tensor("qT", [npairs, 64, S], dt_in,
                        kind="ExternalInput").ap()
    kT = nc.dram_tensor("kT", [npairs, 64, 512 + 64 * NW], dt_in,
                        kind="ExternalInput").ap()
    vs = nc.dram_tensor("vs", [npairs, 128, NCH * 65], dt_in,
                        kind="ExternalInput").ap()
    vw2 = nc.dram_tensor("vw2", [npairs, 112, (NW // 2) * 65], dt_in,
                         kind="ExternalInput").ap()
    out = nc.dram_tensor("out", [npairs, 65, S], f32,
                         kind="ExternalOutput").ap()

    COPY = mybir.ActivationFunctionType.Copy

    with tile.TileContext(nc) as tc, ExitStack() as ctx:
        qk_pool = ctx.enter_context(tc.tile_pool(name="qk", bufs=2))
        v_pool = ctx.enter_context(tc.tile_pool(name="v", bufs=2))
        p_pool = ctx.enter_context(tc.tile_pool(name="p", bufs=2))
        s_pool = ctx.enter_context(tc.tile_pool(name="s", bufs=3, space="PSUM"))
        o_pool = ctx.enter_context(tc.tile_pool(name="o", bufs=2, space="PSUM"))

        def load_tiles(p):
            qt = qk_pool.tile([64, S], dt_in, tag="q")
            nc.sync.dma_start(out=qt, in_=qT[p])
            kt = qk_pool.tile([64, 512 + 64 * NW], dt_in, tag="k")
            nc.sync.dma_start(out=kt, in_=kT[p])
            vst = v_pool.tile([128, NCH * 65], dt_in, tag="vs")
            nc.sync.dma_start(out=vst, in_=vs[p])
            vwt = v_pool.tile([112, (NW // 2) * 65], dt_in, tag="vw")
            nc.sync.dma_start(out=vwt, in_=vw2[p])
            ps = p_pool.tile([128, NCH, S], dt_in, tag="ps")
            pw = p_pool.tile([112, S], dt_in, tag="pw")
            # zero P-window so the cross-window blocks of the stacked
            # window matmuls contribute nothing
            nc.gpsimd.memset(pw, 0.0)
            return dict(p=p, qt=qt, kt=kt, vst=vst, vwt=vwt, ps=ps, pw=pw)

        def pv_step(cx, i):
            # i in 0..31: per q-quarter qg: 4 stripe MMs then 4 window MMs.
            # O'^T accumulates in a [65, 512] quarter; V_aug ones col lands
            # the softmax denominator L in row 64.
            qg, r = i // 8, i % 8
            if r == 0:
                cx["ov" + str(qg)] = o_pool.tile([128, 512], f32, tag="o",
                                                 name=f"ov{cx['p']}_{qg}")
            ov = cx["ov" + str(qg)]
            if r < 4:
                c = r
                nc.tensor.matmul(
                    out=ov[0:65, :],
                    lhsT=cx["vst"][:, c * 65:(c + 1) * 65],
                    rhs=cx["ps"][:, c, qg * 512:(qg + 1) * 512],
                    start=(c == 0), stop=False, skip_group_check=True)
            else:
                j = 4 * qg + (r - 4)
                q0 = (j % 4) * 128
                nc.tensor.matmul(
                    out=ov[0:65, q0:q0 + 128],
                    lhsT=cx["vwt"][:, j * 65:(j + 1) * 65],
                    rhs=cx["pw"][0:112, 128 * j:128 * j + 128],
                    start=False, stop=(r == 7), skip_group_check=True)

        def pv_copy(cx, qg):
            # PSUM -> SBUF staging (DMA cannot read PSUM), all on DVE
            ov = cx["ov" + str(qg)]
            ob = p_pool.tile([65, 512], f32, tag="ob")
            nc.vector.tensor_copy(ob, ov[0:65, :])
            nc.sync.dma_start(
                out=out[cx["p"], :, qg * 512:(qg + 1) * 512], in_=ob)

        ctxs = [load_tiles(0)]
        for p in range(npairs):
            nxt_needed = p + 1 < npairs
            cur = ctxs[p]
            prev = ctxs[p - 1] if p > 0 else None
            # emit, prefetching next pair's tiles after the first round
            qt, kt, ps, pw = cur["qt"], cur["kt"], cur["ps"], cur["pw"]
            for r in range(8):
                c, g = r // 2, r % 2
                st = s_pool.tile([128, 1024], f32, tag="s")
                for u in range(2):
                    q0 = g * 1024 + u * 512
                    nc.tensor.matmul(
                        out=st[:, u * 512:(u + 1) * 512],
                        lhsT=kt[:, c * 128:(c + 1) * 128],
                        rhs=qt[:, q0:q0 + 512],
                        start=True, stop=True)
                po = ps[:, c, g * 1024:(g + 1) * 1024]
                if c == 3 and g == 1:
                    nc.vector.tensor_scalar(
                        out=po.bitcast(i16), in0=st,
                        scalar1=SCH_A, scalar2=SCH_B, op0=MUL, op1=ADD)
                else:
                    nc.scalar.activation(out=po, in_=st,
                                         func=EXP, scale=SCALE)
                if r == 0 and nxt_needed:
                    ctxs.append(load_tiles(p + 1))
                if prev is not None:
                    for i in range(4 * r, 4 * r + 4):
                        pv_step(prev, i)
                    if r % 2 == 1:
                        pv_copy(prev, r // 2)
            for h in range(2):
                sw = s_pool.tile([128, 1024], f32, tag="s")
                for j in range(8 * h, 8 * h + 8):
                    fo = (j - 8 * h) * 128
                    nc.tensor.matmul(
                        out=sw[0:112, fo:fo + 128],
                        lhsT=kt[:, 512 + 128 * j:512 + 128 * j + 112],
                        rhs=qt[:, 128 * j:128 * j + 128],
                        start=True, stop=True)
                sw4 = sw.rearrange("p (j t f) -> p j t f", t=2, f=64)
                pw4 = pw.rearrange("p (j t f) -> p j t f", t=2, f=64)
                nc.vector.tensor_scalar(
                    out=pw4[0:48, 8 * h:8 * h + 8, 0, :].bitcast(i16),
                    in0=sw4[0:48, 0:8, 0, :],
                    scalar1=SCH_A, scalar2=SCH_B, op0=MUL, op1=ADD)
                nc.vector.tensor_scalar(
                    out=pw4[64:112, 8 * h:8 * h + 8, 1, :].bitcast(i16),
                    in0=sw4[64:112, 0:8, 1, :],
                    scalar1=SCH_A, scalar2=SCH_B, op0=MUL, op1=ADD)
        # flush: PV of the last pair
        last = ctxs[-1]
        for i in range(32):
            pv_step(last, i)
            if i % 8 == 7:
                pv_copy(last, i // 8)

    nc.compile()
    return nc


def _get_nc(dt_in_name="float16"):
    if dt_in_name not in _CACHE:
        _CACHE[dt_in_name] = _build(dt_in_name)
    return _CACHE[dt_in_name]


def _prep_inputs(query, key, value, np_dt):
    q = np.asarray(query).reshape(NPAIRS, S, D)
    k = np.asarray(key).reshape(NPAIRS, S, D)
    v = np.asarray(value).reshape(NPAIRS, S, D)
    kr = k[:, _REORDER, :]
    vr = v[:, _REORDER, :]
    qT = np.ascontiguousarray(q.transpose(0, 2, 1)).astype(np_dt)
    # kT: 512 stripe cols, then window-pairs padded to 128 cols each
    # [48 even | 16 zero | 48 odd | 16 zero]
    kTs = kr.transpose(0, 2, 1).astype(np_dt)  # [P, 64, 2048]
    kT = np.zeros((NPAIRS, 64, 512 + 64 * NW), np_dt)
    kT[:, :, 0:512] = kTs[:, :, 0:512]
    kw = kTs[:, :, 512:].reshape(NPAIRS, 64, NW // 2, 2, 48)
    kTw = kT[:, :, 512:].reshape(NPAIRS, 64, NW // 2, 2, 64)
    kTw[:, :, :, :, 0:48] = kw
    va = np.concatenate(
        [vr, np.ones((NPAIRS, S, 1), vr.dtype)], axis=2).astype(np_dt)
    # stripe V_aug: [pair, partition r, chunk c, 65]
    vs = np.ascontiguousarray(
        va[:, :512].reshape(NPAIRS, NCH, 128, 65).transpose(0, 2, 1, 3)
    ).reshape(NPAIRS, 128, NCH * 65)
    # window-pair V_aug: [pair, 112, pairidx i, 65]: rows 0:48 window 2i,
    # rows 64:112 window 2i+1, rows 48:64 zero
    vw = va[:, 512:].reshape(NPAIRS, NW // 2, 2, 48, 65)
    vw2 = np.zeros((NPAIRS, 112, NW // 2, 65), np_dt)
    vw2[:, 0:48] = vw[:, :, 0].transpose(0, 2, 1, 3)
    vw2[:, 64:112] = vw[:, :, 1].transpose(0, 2, 1, 3)
    vw2 = np.ascontiguousarray(vw2).reshape(NPAIRS, 112, (NW // 2) * 65)
    in_maps = []
    for core in range(NCORES):
        sl = slice(core * P_PER_CORE, (core + 1) * P_PER_CORE)
        in_maps.append({"qT": np.ascontiguousarray(qT[sl]),
                        "kT": np.ascontiguousarray(kT[sl]),
                        "vs": np.ascontiguousarray(vs[sl]),
                        "vw2": np.ascontiguousarray(vw2[sl])})
    return in_maps


def _run(query, key, value, dt_in_name="float16", trace=False):
    from concourse.bass_utils import run_bass_kernel_spmd
    nc = _get_nc(dt_in_name)
    in_maps = _prep_inputs(query, key, value, np.float16
                           if dt_in_name == "float16" else np.float32)
    res = run_bass_kernel_spmd(nc, in_maps, list(range(NCORES)), trace=trace)
    o = np.concatenate([res.results[i]["out"] for i in range(NCORES)], axis=0)
    full = (o[:, 0:64, :] / o[:, 64:65, :]).transpose(0, 2, 1).reshape(
        B, H, S, D).astype(np.float32)
    return full, res


def kernel(query, key, value):
    full, _ = _run(np.asarray(query), np.asarray(key), np.asarray(value))
    return full
